# revision 1
# baseline (speedup 1.0000x reference)
"""3-layer GCN (PyG-style GCNConv with self-loops + symmetric norm) on 8
Trainium2 NeuronCores.

Distribution (1D graph partitioning):
  - nodes split into 8 contiguous blocks of 6250 rows, one per core
  - edges partitioned by destination core, sorted by destination node
  - 256x256 weights replicated on every core

Per layer, per core:
  1. GEMM: y_c = h_c @ W.T  (PE transpose of h tiles, then 2 accumulating
     matmuls against W.T blocks)
  2. AllGather y_c -> y_table[50000, 256] (+ row 50000 = layer bias)
  3. message passing for the core's ~106k incoming edges:
     - edges sorted by dst, grouped into 128-node dst chunks, packed into
       128-edge tiles; since every node has a self-loop, any 128
       consecutive sorted edges span <= 128 distinct dst rows
     - per chunk, edges are split by src parity into an EVEN and an ODD
       stream; each stream gathers y[src] via dma_gather (int16 indices,
       stride-2-row table views, up to G*128 rows per instruction)
     - selection matrix selT[e, d] = (dst_local[e] == d) * norm[e] built
       on-chip from an iota compare, then PSUM-accumulated matmuls
       out_chunk += selT.T @ msg
     - bias enters as a reserved edge (slot 0 of each chunk's even
       stream) whose selection column is forced to all-ones by a constant
       mask and whose gathered row is the bias vector (table row 50000)
  4. epilogue: relu(+bias already in PSUM), residual add (layers 1,2),
     write h rows back to DRAM
"""

import math
import os

import numpy as np

import concourse.bass as bass
import concourse.mybir as mybir
import concourse.tile as tile
from concourse import bacc
from concourse.bass_utils import run_bass_kernel_spmd
from concourse.masks import make_identity

F32 = mybir.dt.float32
I16 = mybir.dt.int16

N_NODES = 50000
HID = 256
NCORES = 8
NPC = N_NODES // NCORES          # 6250 nodes per core
NCHUNK = math.ceil(NPC / 128)    # 49 dst chunks per core
G = 8                            # edge tiles per gather instruction (dma_gather tops out at 1024 idxs)
PAD_DST = 255.0                  # dst_local sentinel that matches no iota lane
NLAYERS = 3
DEBUG_GEMM_ONLY = False
NSWDGE_QUEUES = 4                # parallel SWDGE descriptor-gen queues
MM_DT = mybir.dt.float32  # float32r would be 4x PE rate but ~2e-4 rel err

_cache = {}


def _pack_stream(flat_idx, flat_dst, flat_nrm, NG):
    """flat_* are [NG*G*128] slot arrays in (tile, slot) order.

    Returns (idxT [NG*128, G*8] int16, dstT [NG*128, G] f32,
    nrmT [NG*128, G] f32) in the per-gather-group on-chip layouts.
    """
    dstT = (
        flat_dst.reshape(NG, G, 128).transpose(0, 2, 1).reshape(NG * 128, G)
    )
    nrmT = (
        flat_nrm.reshape(NG, G, 128).transpose(0, 2, 1).reshape(NG * 128, G)
    )
    idxT = np.zeros((NG * 128, G * 8), dtype=np.int16)
    vals = flat_idx.reshape(NG, G * 128)
    for g in range(NG):
        a16 = vals[g].reshape(G * 8, 16).T  # [16, G*8]; slot i at [i%16, i//16]
        idxT[g * 128 : (g + 1) * 128] = np.tile(a16, (8, 1))
    # pack: per row = [G*8 int16 idx | G f32 dst | G f32 nrm] viewed as int32
    meta = np.zeros((NG * 128, G * 4 + G + G), dtype=np.int32)
    meta[:, : G * 4] = idxT.view(np.int32)
    meta[:, G * 4 : G * 5] = dstT.astype(np.float32).view(np.int32)
    meta[:, G * 5 : G * 6] = nrmT.astype(np.float32).view(np.int32)
    return (meta,)


def _preprocess(edge_index):
    """Edge partitioning by destination + per-core parity-stream layouts."""
    src = np.asarray(edge_index[0], dtype=np.int64)
    dst = np.asarray(edge_index[1], dtype=np.int64)
    loops = np.arange(N_NODES, dtype=np.int64)
    s = np.concatenate([src, loops])
    d = np.concatenate([dst, loops])
    deg = np.bincount(d, minlength=N_NODES).astype(np.float32)
    dinv = (1.0 / np.sqrt(deg)).astype(np.float32)
    norm = (dinv[s] * dinv[d]).astype(np.float32)

    # per (core, chunk, parity) edge lists
    edges = []  # [core][chunk] -> (even(src,dstl,nrm), odd(...))
    cntE = np.zeros((NCORES, NCHUNK), dtype=np.int64)
    cntO = np.zeros((NCORES, NCHUNK), dtype=np.int64)
    for c in range(NCORES):
        lo = c * NPC
        m = (d >= lo) & (d < lo + NPC)
        cs, cd, cn = s[m], (d[m] - lo), norm[m]
        order = np.argsort(cd, kind="stable")
        cs, cd, cn = cs[order], cd[order], cn[order]
        bounds = np.searchsorted(cd, np.arange(0, NCHUNK + 1) * 128)
        rows = []
        for ch in range(NCHUNK):
            a, b = bounds[ch], bounds[ch + 1]
            es, ed, en = cs[a:b], cd[a:b] - ch * 128, cn[a:b]
            ms = es + es // NPC  # row in the allgathered [N+NCORES] table
            pe = (ms % 2) == 0
            ev = (ms[pe] // 2, ed[pe], en[pe])
            od = (ms[~pe] // 2, ed[~pe], en[~pe])
            rows.append((ev, od))
            cntE[c, ch] = pe.sum() + 1  # +1 bias edge
            cntO[c, ch] = (~pe).sum()
        edges.append(rows)

    TE = [int(np.ceil(cntE[:, ch].max() / 128)) for ch in range(NCHUNK)]
    TO = [int(np.ceil(cntO[:, ch].max() / 128)) for ch in range(NCHUNK)]
    tilesE, tilesO = int(np.sum(TE)), int(np.sum(TO))
    NGE, NGO = math.ceil(tilesE / G), math.ceil(tilesO / G)
    startE = np.concatenate([[0], np.cumsum(TE)]).astype(int)
    startO = np.concatenate([[0], np.cumsum(TO)]).astype(int)


    per_core = []
    for c in range(NCORES):
        fiE = np.zeros(NGE * G * 128, dtype=np.int64)  # pad idx: even row 0
        fdE = np.full(NGE * G * 128, PAD_DST, dtype=np.float32)
        fnE = np.zeros(NGE * G * 128, dtype=np.float32)
        fiO = np.zeros(NGO * G * 128, dtype=np.int64)  # pad idx: odd row 0
        fdO = np.full(NGO * G * 128, PAD_DST, dtype=np.float32)
        fnO = np.zeros(NGO * G * 128, dtype=np.float32)
        for ch in range(NCHUNK):
            (eis, eds, ens), (ois, ods, ons) = edges[c][ch]
            p0 = startE[ch] * 128
            L = len(eis) + 1
            fiE[p0 : p0 + L] = np.concatenate([[NPC // 2], eis])
            fdE[p0 + 1 : p0 + L] = eds
            fnE[p0 + 1 : p0 + L] = ens
            p0 = startO[ch] * 128
            L = len(ois)
            fiO[p0 : p0 + L] = ois
            fdO[p0 : p0 + L] = ods
            fnO[p0 : p0 + L] = ons
        per_core.append(
            _pack_stream(fiE, fdE, fnE, NGE) + _pack_stream(fiO, fdO, fnO, NGO)
        )

    sched = (tuple(TE), tuple(TO), tilesE, tilesO, NGE, NGO)
    return sched, per_core


def _build(sched, nlayers=3):
    TE, TO, tilesE, tilesO, NGE, NGO = sched
    nc = bacc.Bacc(
        "TRN2",
        target_bir_lowering=False,
        debug=False,
        num_devices=NCORES,
        num_swdge_queues=NSWDGE_QUEUES,
    )
    x_ap = nc.dram_tensor("x", [NPC, HID], F32, kind="ExternalInput").ap()
    wts = nc.dram_tensor(
        "wts", [2 * nlayers, 128, HID], MM_DT, kind="ExternalInput"
    ).ap()
    bias = nc.dram_tensor("bias", [nlayers, HID], MM_DT, kind="ExternalInput").ap()
    consts = nc.dram_tensor("consts", [128, 256], F32, kind="ExternalInput").ap()
    I32 = mybir.dt.int32
    metE = nc.dram_tensor(
        "metE", [NGE * 128, G * 6], I32, kind="ExternalInput"
    ).ap()
    metO = nc.dram_tensor(
        "metO", [NGO * 128, G * 6], I32, kind="ExternalInput"
    ).ap()
    out_ap = nc.dram_tensor("out", [NPC, HID], F32, kind="ExternalOutput").ap()

    with tile.TileContext(nc) as tc:
        with tc.tile_pool(name="const", bufs=1) as cpool, \
             tc.tile_pool(name="hpool", bufs=1) as hpool, \
             tc.tile_pool(name="work", bufs=3) as work, \
             tc.tile_pool(name="meta", bufs=8) as meta, \
             tc.tile_pool(name="msgp", bufs=8) as msgp, \
             tc.tile_pool(name="eqp", bufs=8) as eqp, \
             tc.tile_pool(name="ptp", bufs=2, space="PSUM") as ptp, \
             tc.tile_pool(name="ypp", bufs=2, space="PSUM") as ypp, \
             tc.tile_pool(name="psp", bufs=4, space="PSUM") as psp, \
             tc.tile_pool(name="dram", bufs=1, space="DRAM") as dram:

            identity = cpool.tile([128, 128], F32)
            make_identity(nc, identity[:])
            cst = cpool.tile([128, 256], F32)
            nc.sync.dma_start(out=cst[:], in_=consts[:])
            iota_sb = cst[:, 0:128]
            mask_sb = cst[:, 128:256]

            wt_sb = cpool.tile([128, 2 * nlayers * HID], MM_DT)
            for i in range(2 * nlayers):
                nc.sync.dma_start(
                    out=wt_sb[:, i * HID : (i + 1) * HID], in_=wts[i]
                )

            # h lives in SBUF, one tile per 128-node chunk, updated in place
            h_sb = [
                hpool.tile([128, HID], F32, tag=f"h{c}", name=f"h_sb{c}")
                for c in range(NCHUNK)
            ]
            for c in range(NCHUNK):
                rows = min(128, NPC - c * 128)
                nc.sync.dma_start(
                    out=h_sb[c][:rows], in_=x_ap[c * 128 : c * 128 + rows, :]
                )

            y_cs = [
                dram.tile([NPC + 1, HID], MM_DT, name=f"y_c{i}")
                for i in range(nlayers)
            ]
            y_tables = [
                dram.tile(
                    [(NPC + 1) * NCORES, HID],
                    MM_DT,
                    addr_space="Shared",
                    name=f"y_table{i}",
                )
                for i in range(nlayers)
            ]
            for l in range(nlayers):
                nc.sync.dma_start(
                    out=y_cs[l][NPC : NPC + 1, :], in_=bias[l : l + 1, :]
                )

            def gemm_chunk(l, c):
                """y_cs[l] rows of chunk c = h_sb[c] @ W_l.T"""
                rows = min(128, NPC - c * 128)
                hT = work.tile([128, HID], MM_DT, tag="hT", name="hT")
                for k in range(2):
                    pt = ptp.tile([128, 128], F32, tag="pt", name="pt")
                    nc.tensor.transpose(
                        out=pt[:, :rows],
                        in_=h_sb[c][:rows, k * 128 : (k + 1) * 128],
                        identity=identity[:rows, :rows],
                    )
                    nc.vector.tensor_copy(
                        out=hT[:, k * 128 : k * 128 + rows], in_=pt[:, :rows]
                    )
                yp = ypp.tile([128, HID], F32, tag="yp", name="yp")
                for k in range(2):
                    nc.tensor.matmul(
                        out=yp[:rows, :],
                        lhsT=hT[:, k * 128 : k * 128 + rows],
                        rhs=wt_sb[:, (2 * l + k) * HID : (2 * l + k + 1) * HID],
                        start=(k == 0),
                        stop=(k == 1),
                    )
                y_sb = work.tile([128, HID], MM_DT, tag="y_sb", name="y_sb")
                nc.vector.tensor_copy(out=y_sb[:rows], in_=yp[:rows, :])
                nc.sync.dma_start(
                    out=y_cs[l][c * 128 : c * 128 + rows, :], in_=y_sb[:rows]
                )

            for c in range(NCHUNK):
                gemm_chunk(0, c)

            for l in range(nlayers):
                y_table = y_tables[l]
                stream_info = {
                    "E": (metE, tilesE, y_table[0::2, :]),
                    "O": (metO, tilesO, y_table[1::2, :]),
                }

                nc.gpsimd.collective_compute(
                    "AllGather",
                    mybir.AluOpType.bypass,
                    replica_groups=[list(range(NCORES))],
                    ins=[y_cs[l][:].opt()],
                    outs=[y_table[:].opt()],
                )

                pos = {"E": 0, "O": 0}
                bufs = {}
                for ci in range(NCHUNK):
                    crows = min(128, NPC - ci * 128)
                    ntot = TE[ci] + TO[ci]
                    ps = psp.tile([128, HID], F32, tag="ps", name="ps")
                    jj = 0
                    for sname, T_s in (("E", TE), ("O", TO)):
                        met_d, tiles_s, view = stream_info[sname]
                        for t in range(T_s[ci]):
                            st = pos[sname]
                            g, col = divmod(st, G)
                            if col == 0:
                                rem = min(G, tiles_s - g * G)
                                met_sb = meta.tile(
                                    [128, G * 6], I32, tag="met_sb", name="met_sb"
                                )
                                nc.sync.dma_start(
                                    out=met_sb[:],
                                    in_=met_d[g * 128 : (g + 1) * 128, :],
                                )
                                idx_sb = met_sb[:, : G * 4].bitcast(I16)
                                dst_sb = met_sb[:, G * 4 : G * 5].bitcast(F32)
                                nrm_sb = met_sb[:, G * 5 : G * 6].bitcast(F32)
                                msg = msgp.tile(
                                    [128, G * HID], MM_DT, tag="msg", name="msg"
                                )
                                nc.gpsimd.dma_gather(
                                    out_ap=msg[:, : rem * HID].rearrange(
                                        "p (g d) -> p g d", g=rem
                                    ),
                                    in_ap=view,
                                    idxs_ap=idx_sb[:, : rem * 8],
                                    num_idxs=rem * 128,
                                    num_idxs_reg=rem * 128,
                                    elem_size=HID,
                                    elem_step=2 * HID,
                                    queue_num=(g + (0 if sname == "E" else 2))
                                    % NSWDGE_QUEUES,
                                )
                                eq = eqp.tile(
                                    [128, G * 128], MM_DT, tag="eq", name="eq"
                                )
                                eq3 = eq[:, : rem * 128].rearrange(
                                    "p (g d) -> p g d", g=rem
                                )
                                nc.vector.tensor_tensor(
                                    out=eq3,
                                    in0=dst_sb[:, :rem, None].to_broadcast(
                                        (128, rem, 128)
                                    ),
                                    in1=iota_sb[:, None, :].to_broadcast(
                                        (128, rem, 128)
                                    ),
                                    op=mybir.AluOpType.is_equal,
                                )
                                nc.vector.tensor_tensor(
                                    out=eq3,
                                    in0=eq3,
                                    in1=nrm_sb[:, :rem, None].to_broadcast(
                                        (128, rem, 128)
                                    ),
                                    op=mybir.AluOpType.mult,
                                )
                                bufs[sname] = (msg, eq)
                            msg, eq = bufs[sname]
                            if sname == "E" and t == 0:
                                # bias edge: force its sel column to ones
                                nc.vector.tensor_tensor(
                                    out=eq[:, col * 128 : (col + 1) * 128],
                                    in0=eq[:, col * 128 : (col + 1) * 128],
                                    in1=mask_sb,
                                    op=mybir.AluOpType.add,
                                )
                            nc.tensor.matmul(
                                out=ps[:, :],
                                lhsT=eq[:, col * 128 : (col + 1) * 128],
                                rhs=msg[:, col * HID : (col + 1) * HID],
                                start=(jj == 0),
                                stop=(jj == ntot - 1),
                            )
                            pos[sname] += 1
                            jj += 1
                    # epilogue: relu (+bias in psum), residual, h update
                    if l == 0:
                        nc.scalar.activation(
                            out=h_sb[ci][:crows],
                            in_=ps[:crows, :],
                            func=mybir.ActivationFunctionType.Relu,
                        )
                    else:
                        o_sb = work.tile([128, HID], F32, tag="o_sb", name="o_sb")
                        nc.scalar.activation(
                            out=o_sb[:crows],
                            in_=ps[:crows, :],
                            func=mybir.ActivationFunctionType.Relu,
                        )
                        if l < nlayers - 1:
                            nc.vector.tensor_add(
                                out=h_sb[ci][:crows],
                                in0=o_sb[:crows],
                                in1=h_sb[ci][:crows],
                            )
                        else:
                            nc.vector.tensor_add(
                                out=o_sb[:crows],
                                in0=o_sb[:crows],
                                in1=h_sb[ci][:crows],
                            )
                            nc.sync.dma_start(
                                out=out_ap[ci * 128 : ci * 128 + crows, :],
                                in_=o_sb[:crows],
                            )
                    if l + 1 < nlayers:
                        gemm_chunk(l + 1, ci)

    nc.compile()
    return nc


def _consts_array():
    consts = np.zeros((128, 256), dtype=np.float32)
    consts[:, 0:128] = np.arange(128, dtype=np.float32)[None, :]
    consts[0, 128:256] = 1.0
    return consts


def kernel(x, edge_index, W0, b0, W1, b1, W2, b2):
    x = np.asarray(x, dtype=np.float32)
    edge_index = np.asarray(edge_index)
    Ws = [np.asarray(w, dtype=np.float32) for w in (W0, W1, W2)]
    bs = [np.asarray(b, dtype=np.float32) for b in (b0, b1, b2)]

    sched, per_core = _preprocess(edge_index)

    key = (sched, NLAYERS, DEBUG_GEMM_ONLY)
    if key not in _cache:
        _cache[key] = _build(sched, nlayers=NLAYERS)
    nc = _cache[key]

    wts = np.stack(
        [w.T[k * 128 : (k + 1) * 128, :] for w in Ws for k in range(2)]
    ).astype(np.float32)
    bias_arr = np.stack(bs).astype(np.float32)
    consts = _consts_array()

    in_maps = []
    for c in range(NCORES):
        mE, mO = per_core[c]
        in_maps.append(
            {
                "x": np.ascontiguousarray(x[c * NPC : (c + 1) * NPC]),
                "wts": wts,
                "bias": bias_arr,
                "consts": consts,
                "metE": mE,
                "metO": mO,
            }
        )

    trace = bool(int(os.environ.get("GCN_TRACE", "0")))
    res = run_bass_kernel_spmd(
        nc, in_maps, core_ids=list(range(NCORES)), trace=trace
    )
    if trace:
        kernel.last_exec_time_ns = res.exec_time_ns
        kernel.last_results = res
    out = np.concatenate([res.results[c]["out"] for c in range(NCORES)], axis=0)
    return out



# revision 7
# speedup vs baseline: 1.5480x; 1.5480x over previous
"""3-layer GCN (PyG-style GCNConv with self-loops + symmetric norm) on 8
Trainium2 NeuronCores.

Distribution (1D graph partitioning):
  - nodes split into 8 contiguous blocks of 6250 rows, one per core
  - edges partitioned by destination core, sorted by destination node
  - 256x256 weights replicated on every core

Per layer, per core (fp16 data path, fp32 accumulation):
  1. GEMM: y_c = (h_c @ W.T) * dinv[src] in fp16 (PE transpose of h tiles, 2
     accumulating matmuls against W.T blocks, dinv fold + fp16 cast on the
     Scalar engine), staged into local DRAM y_c
  2. AllGather y_c -> y_table[8*(NPC+1), 256] fp16 (+ row NPC of each shard
     = layer bias)
  3. message passing for the core's ~106k incoming edges:
     - edges sorted by dst, grouped into 128-node dst chunks, packed into
       128-edge tiles; since every node has a self-loop, any 128
       consecutive sorted edges span <= 128 distinct dst rows
     - per chunk, edges are split by src parity into an EVEN and an ODD
       stream; each stream gathers ytilde[src] via dma_gather (int16
       indices, stride-2-row table views, up to G*128 rows per instruction)
     - selection matrix selT[e, d] = (dst_local[e] == d) built on-chip with
       ONE DVE is_equal per gather group (norm is factored out: dinv[src]
       lives in the table rows, dinv[dst] is applied as the epilogue
       activation scale), then PSUM-accumulated fp16 matmuls
       out_chunk += selT.T @ msg
     - bias enters as a reserved edge (slot 0 of each chunk's even stream)
       whose selection column is set to sqrt(deg[dst]) (cancelling the
       epilogue dinv[dst] scale) and whose gathered row is the bias vector
  4. epilogue: relu(dinv[dst] * psum), residual add (layers 1,2), write h
     rows back to DRAM
"""

import math
import os

import numpy as np

import concourse.bass as bass
import concourse.mybir as mybir
import concourse.tile as tile
from concourse import bacc
from concourse.bass_utils import run_bass_kernel_spmd
from concourse.masks import make_identity

F32 = mybir.dt.float32
F16 = mybir.dt.float16
I16 = mybir.dt.int16
I32 = mybir.dt.int32

N_NODES = 50000
HID = 256
NCORES = 8
NPC = N_NODES // NCORES          # 6250 nodes per core
NCHUNK = math.ceil(NPC / 128)    # 49 dst chunks per core
G = 8                            # edge tiles per gather instruction
PAD_DST = 255.0                  # dst_local sentinel that matches no iota lane
NLAYERS = 3
NSWDGE_QUEUES = 4                # parallel SWDGE descriptor-gen queues
MM_DT = F16                      # message/eq/weight dtype (PSUM accum is f32)
MW = G * 4 + G // 2              # meta int32 words: idx int16 x8G + dst f16 xG

_cache = {}


def _pack_stream(flat_idx, flat_dst, NG):
    """flat_* are [NG*G*128] slot arrays in (tile, slot) order.

    Returns packed meta [NG*128, MW] int32: per row
    [G*8 int16 idx | G f16 dst].
    """
    dstT = (
        flat_dst.reshape(NG, G, 128).transpose(0, 2, 1).reshape(NG * 128, G)
    )
    idxT = np.zeros((NG * 128, G * 8), dtype=np.int16)
    vals = flat_idx.reshape(NG, G * 128)
    for g in range(NG):
        a16 = vals[g].reshape(G * 8, 16).T  # [16, G*8]; slot i at [i%16, i//16]
        idxT[g * 128 : (g + 1) * 128] = np.tile(a16, (8, 1))
    meta = np.zeros((NG * 128, MW), dtype=np.int32)
    meta[:, : G * 4] = idxT.view(np.int32)
    meta[:, G * 4 :] = dstT.astype(np.float16).reshape(-1, G).view(np.int32)
    return (meta,)


def _preprocess(edge_index):
    """Edge partitioning by destination + per-core parity-stream layouts."""
    src = np.asarray(edge_index[0], dtype=np.int64)
    dst = np.asarray(edge_index[1], dtype=np.int64)
    loops = np.arange(N_NODES, dtype=np.int64)
    s = np.concatenate([src, loops])
    d = np.concatenate([dst, loops])
    deg = np.bincount(d, minlength=N_NODES).astype(np.float32)
    dinv = (1.0 / np.sqrt(deg)).astype(np.float32)

    edges = []  # [core][chunk] -> (even(src,dstl), odd(...))
    cntE = np.zeros((NCORES, NCHUNK), dtype=np.int64)
    cntO = np.zeros((NCORES, NCHUNK), dtype=np.int64)
    for c in range(NCORES):
        lo = c * NPC
        m = (d >= lo) & (d < lo + NPC)
        cs, cd = s[m], (d[m] - lo)
        order = np.argsort(cd, kind="stable")
        cs, cd = cs[order], cd[order]
        bounds = np.searchsorted(cd, np.arange(0, NCHUNK + 1) * 128)
        rows = []
        for ch in range(NCHUNK):
            a, b = bounds[ch], bounds[ch + 1]
            es, ed = cs[a:b], cd[a:b] - ch * 128
            ms = es + es // NPC  # row in the allgathered [8*(NPC+1)] table
            pe = (ms % 2) == 0
            ev = (ms[pe] // 2, ed[pe])
            od = (ms[~pe] // 2, ed[~pe])
            rows.append((ev, od))
            cntE[c, ch] = pe.sum() + 1  # +1 bias edge
            cntO[c, ch] = (~pe).sum()
        edges.append(rows)

    TE = [int(np.ceil(cntE[:, ch].max() / 128)) for ch in range(NCHUNK)]
    TO = [int(np.ceil(cntO[:, ch].max() / 128)) for ch in range(NCHUNK)]
    tilesE, tilesO = int(np.sum(TE)), int(np.sum(TO))
    NGE, NGO = math.ceil(tilesE / G), math.ceil(tilesO / G)
    startE = np.concatenate([[0], np.cumsum(TE)]).astype(int)
    startO = np.concatenate([[0], np.cumsum(TO)]).astype(int)

    per_core = []
    for c in range(NCORES):
        fiE = np.zeros(NGE * G * 128, dtype=np.int64)  # pad idx: even row 0
        fdE = np.full(NGE * G * 128, PAD_DST, dtype=np.float32)
        fiO = np.zeros(NGO * G * 128, dtype=np.int64)  # pad idx: odd row 0
        fdO = np.full(NGO * G * 128, PAD_DST, dtype=np.float32)
        for ch in range(NCHUNK):
            (eis, eds), (ois, ods) = edges[c][ch]
            p0 = startE[ch] * 128
            L = len(eis) + 1
            fiE[p0 : p0 + L] = np.concatenate([[NPC // 2], eis])
            fdE[p0 + 1 : p0 + L] = eds
            p0 = startO[ch] * 128
            L = len(ois)
            fiO[p0 : p0 + L] = ois
            fdO[p0 : p0 + L] = ods
        # dinv rows for this core's nodes, [128, NCHUNK] chunk-column layout
        dv = np.zeros(NCHUNK * 128, dtype=np.float32)
        dv[:NPC] = dinv[c * NPC : (c + 1) * NPC]
        dcol = dv.reshape(NCHUNK, 128).T.copy()  # [128, NCHUNK] f32
        # sqrt(deg) rows (bias column values), [1, NCHUNK*128]
        sq = np.zeros(NCHUNK * 128, dtype=np.float16)
        sq[:NPC] = np.sqrt(deg[c * NPC : (c + 1) * NPC]).astype(np.float16)
        per_core.append(
            _pack_stream(fiE, fdE, NGE)
            + _pack_stream(fiO, fdO, NGO)
            + (dcol, sq.reshape(1, -1))
        )

    sched = (tuple(TE), tuple(TO), tilesE, tilesO, NGE, NGO)
    return sched, per_core


def _build(sched, nlayers=3):
    TE, TO, tilesE, tilesO, NGE, NGO = sched
    nc = bacc.Bacc(
        "TRN2",
        target_bir_lowering=False,
        debug=False,
        num_devices=NCORES,
        num_swdge_queues=NSWDGE_QUEUES,
    )
    x_ap = nc.dram_tensor("x", [NPC, HID], F32, kind="ExternalInput").ap()
    wts = nc.dram_tensor(
        "wts", [2 * nlayers, 128, HID], MM_DT, kind="ExternalInput"
    ).ap()
    bias = nc.dram_tensor("bias", [nlayers, HID], MM_DT, kind="ExternalInput").ap()
    consts = nc.dram_tensor("consts", [128, 64], I32, kind="ExternalInput").ap()
    dinvc = nc.dram_tensor("dinvc", [128, NCHUNK], F32, kind="ExternalInput").ap()
    sqdeg = nc.dram_tensor(
        "sqdeg", [1, NCHUNK * 64], I32, kind="ExternalInput"
    ).ap()
    metE = nc.dram_tensor("metE", [NGE * 128, MW], I32, kind="ExternalInput").ap()
    metO = nc.dram_tensor("metO", [NGO * 128, MW], I32, kind="ExternalInput").ap()
    out_ap = nc.dram_tensor("out", [NPC, HID], F32, kind="ExternalOutput").ap()

    with tile.TileContext(nc) as tc:
        with tc.tile_pool(name="const", bufs=1) as cpool, \
             tc.tile_pool(name="hpool", bufs=1) as hpool, \
             tc.tile_pool(name="work", bufs=3) as work, \
             tc.tile_pool(name="meta", bufs=8) as meta, \
             tc.tile_pool(name="msgp", bufs=8) as msgp, \
             tc.tile_pool(name="eqp", bufs=8) as eqp, \
             tc.tile_pool(name="ptp", bufs=2, space="PSUM") as ptp, \
             tc.tile_pool(name="ypp", bufs=2, space="PSUM") as ypp, \
             tc.tile_pool(name="psp", bufs=4, space="PSUM") as psp, \
             tc.tile_pool(name="dram", bufs=1, space="DRAM") as dram:

            identity = cpool.tile([128, 128], F32)
            make_identity(nc, identity[:])
            cst = cpool.tile([128, 64], I32)
            nc.sync.dma_start(out=cst[:], in_=consts[:])
            iota_sb = cst[:, 0:64].bitcast(F16)    # [128,128] rows = 0..127
            dv_sb = cpool.tile([128, NCHUNK], F32)
            nc.sync.dma_start(out=dv_sb[:], in_=dinvc[:])
            sq_sb = cpool.tile([1, NCHUNK * 64], I32)
            nc.sync.dma_start(out=sq_sb[:], in_=sqdeg[:])
            sq16 = sq_sb.bitcast(F16)  # [1, NCHUNK*128] f16

            wt_sb = cpool.tile([128, 2 * nlayers * HID], MM_DT)
            for i in range(2 * nlayers):
                nc.sync.dma_start(
                    out=wt_sb[:, i * HID : (i + 1) * HID], in_=wts[i]
                )

            # h lives in SBUF, one tile per 128-node chunk, updated in place
            h_sb = [
                hpool.tile([128, HID], F32, tag=f"h{c}", name=f"h_sb{c}")
                for c in range(NCHUNK)
            ]
            for c in range(NCHUNK):
                rows = min(128, NPC - c * 128)
                nc.sync.dma_start(
                    out=h_sb[c][:rows], in_=x_ap[c * 128 : c * 128 + rows, :]
                )

            y_cs = [
                dram.tile([NPC + 1, HID], MM_DT, name=f"y_c{i}")
                for i in range(nlayers)
            ]
            y_tables = [
                dram.tile(
                    [(NPC + 1) * NCORES, HID],
                    MM_DT,
                    addr_space="Shared",
                    name=f"y_table{i}",
                )
                for i in range(nlayers)
            ]
            for l in range(nlayers):
                nc.sync.dma_start(
                    out=y_cs[l][NPC : NPC + 1, :], in_=bias[l : l + 1, :]
                )

            def gemm_chunk(l, c):
                """y_cs[l] rows of chunk c = dinv * (h_sb[c] @ W_l.T)"""
                rows = min(128, NPC - c * 128)
                hT = work.tile([128, HID], MM_DT, tag="hT", name="hT")
                for k in range(2):
                    pt = ptp.tile([128, 128], F32, tag="pt", name="pt")
                    nc.tensor.transpose(
                        out=pt[:, :rows],
                        in_=h_sb[c][:rows, k * 128 : (k + 1) * 128],
                        identity=identity[:rows, :rows],
                    )
                    nc.scalar.activation(
                        out=hT[:, k * 128 : k * 128 + rows],
                        in_=pt[:, :rows],
                        func=mybir.ActivationFunctionType.Identity,
                    )
                yp = ypp.tile([128, HID], F32, tag="yp", name="yp")
                for k in range(2):
                    nc.tensor.matmul(
                        out=yp[:rows, :],
                        lhsT=hT[:, k * 128 : k * 128 + rows],
                        rhs=wt_sb[:, (2 * l + k) * HID : (2 * l + k + 1) * HID],
                        start=(k == 0),
                        stop=(k == 1),
                    )
                y_sb = work.tile([128, HID], MM_DT, tag="y_sb", name="y_sb")
                nc.scalar.activation(
                    out=y_sb[:rows],
                    in_=yp[:rows, :],
                    func=mybir.ActivationFunctionType.Identity,
                    scale=dv_sb[:rows, c : c + 1],
                )
                nc.sync.dma_start(
                    out=y_cs[l][c * 128 : c * 128 + rows, :], in_=y_sb[:rows]
                )

            for c in range(NCHUNK):
                gemm_chunk(0, c)

            for l in range(nlayers):
                y_table = y_tables[l]
                stream_info = {
                    "E": (metE, tilesE, y_table[0::2, :]),
                    "O": (metO, tilesO, y_table[1::2, :]),
                }

                nc.gpsimd.collective_compute(
                    "AllGather",
                    mybir.AluOpType.bypass,
                    replica_groups=[list(range(NCORES))],
                    ins=[y_cs[l][:].opt()],
                    outs=[y_table[:].opt()],
                )

                pos = {"E": 0, "O": 0}
                bufs = {}
                for ci in range(NCHUNK):
                    crows = min(128, NPC - ci * 128)
                    ntot = TE[ci] + TO[ci]
                    ps = psp.tile([128, HID], F32, tag="ps", name="ps")
                    jj = 0
                    for sname, T_s in (("E", TE), ("O", TO)):
                        met_d, tiles_s, view = stream_info[sname]
                        for t in range(T_s[ci]):
                            st = pos[sname]
                            g, col = divmod(st, G)
                            if col == 0:
                                rem = min(G, tiles_s - g * G)
                                met_sb = meta.tile(
                                    [128, MW], I32, tag="met_sb", name="met_sb"
                                )
                                nc.sync.dma_start(
                                    out=met_sb[:],
                                    in_=met_d[g * 128 : (g + 1) * 128, :],
                                )
                                idx_sb = met_sb[:, : G * 4].bitcast(I16)
                                dst_sb = met_sb[:, G * 4 :].bitcast(F16)
                                msg = msgp.tile(
                                    [128, G * HID], MM_DT, tag="msg", name="msg"
                                )
                                nc.gpsimd.dma_gather(
                                    out_ap=msg[:, : rem * HID].rearrange(
                                        "p (g d) -> p g d", g=rem
                                    ),
                                    in_ap=view,
                                    idxs_ap=idx_sb[:, : rem * 8],
                                    num_idxs=rem * 128,
                                    num_idxs_reg=rem * 128,
                                    elem_size=HID,
                                    elem_step=2 * HID,
                                    queue_num=(g + (0 if sname == "E" else 2))
                                    % NSWDGE_QUEUES,
                                )
                                eq = eqp.tile(
                                    [128, G * 128], MM_DT, tag="eq", name="eq"
                                )
                                eq3 = eq[:, : rem * 128].rearrange(
                                    "p (g d) -> p g d", g=rem
                                )
                                nc.vector.tensor_tensor(
                                    out=eq3,
                                    in0=dst_sb[:, :rem, None].to_broadcast(
                                        (128, rem, 128)
                                    ),
                                    in1=iota_sb[:, None, :].to_broadcast(
                                        (128, rem, 128)
                                    ),
                                    op=mybir.AluOpType.is_equal,
                                )
                                bufs[sname] = (msg, eq)
                            msg, eq = bufs[sname]
                            if sname == "E" and t == 0:
                                # bias edge: its sel column = sqrt(deg[dst])
                                nc.vector.tensor_tensor(
                                    out=eq[0:1, col * 128 : col * 128 + crows],
                                    in0=eq[0:1, col * 128 : col * 128 + crows],
                                    in1=sq16[0:1, ci * 128 : ci * 128 + crows],
                                    op=mybir.AluOpType.add,
                                )
                            nc.tensor.matmul(
                                out=ps[:, :],
                                lhsT=eq[:, col * 128 : (col + 1) * 128],
                                rhs=msg[:, col * HID : (col + 1) * HID],
                                start=(jj == 0),
                                stop=(jj == ntot - 1),
                            )
                            pos[sname] += 1
                            jj += 1
                    # epilogue: relu(dinv * psum), residual, h update
                    if l == 0:
                        nc.scalar.activation(
                            out=h_sb[ci][:crows],
                            in_=ps[:crows, :],
                            func=mybir.ActivationFunctionType.Relu,
                            scale=dv_sb[:crows, ci : ci + 1],
                        )
                    else:
                        o_sb = work.tile([128, HID], F32, tag="o_sb", name="o_sb")
                        nc.scalar.activation(
                            out=o_sb[:crows],
                            in_=ps[:crows, :],
                            func=mybir.ActivationFunctionType.Relu,
                            scale=dv_sb[:crows, ci : ci + 1],
                        )
                        if l < nlayers - 1:
                            nc.vector.tensor_add(
                                out=h_sb[ci][:crows],
                                in0=o_sb[:crows],
                                in1=h_sb[ci][:crows],
                            )
                        else:
                            nc.vector.tensor_add(
                                out=o_sb[:crows],
                                in0=o_sb[:crows],
                                in1=h_sb[ci][:crows],
                            )
                            nc.sync.dma_start(
                                out=out_ap[ci * 128 : ci * 128 + crows, :],
                                in_=o_sb[:crows],
                            )
                    if l + 1 < nlayers:
                        gemm_chunk(l + 1, ci)

    nc.compile()
    return nc


def _consts_array():
    iota = np.tile(np.arange(128, dtype=np.float16)[None, :], (128, 1))
    return iota.view(np.int32)  # [128, 64] i32


def kernel(x, edge_index, W0, b0, W1, b1, W2, b2):
    x = np.asarray(x, dtype=np.float32)
    edge_index = np.asarray(edge_index)
    Ws = [np.asarray(w, dtype=np.float32) for w in (W0, W1, W2)]
    bs = [np.asarray(b, dtype=np.float32) for b in (b0, b1, b2)]

    sched, per_core = _preprocess(edge_index)

    key = (sched, NLAYERS)
    if key not in _cache:
        _cache[key] = _build(sched, nlayers=NLAYERS)
    nc = _cache[key]

    wts = np.stack(
        [w.T[k * 128 : (k + 1) * 128, :] for w in Ws for k in range(2)]
    ).astype(np.float16)
    bias_arr = np.stack(bs).astype(np.float16)
    consts = _consts_array()

    in_maps = []
    for c in range(NCORES):
        mE, mO, dcol, sq = per_core[c]
        in_maps.append(
            {
                "x": np.ascontiguousarray(x[c * NPC : (c + 1) * NPC]),
                "wts": wts,
                "bias": bias_arr,
                "consts": consts,
                "dinvc": dcol,
                "sqdeg": np.ascontiguousarray(sq).view(np.int32),
                "metE": mE,
                "metO": mO,
            }
        )

    trace = bool(int(os.environ.get("GCN_TRACE", "0")))
    res = run_bass_kernel_spmd(
        nc, in_maps, core_ids=list(range(NCORES)), trace=trace
    )
    if trace:
        kernel.last_exec_time_ns = res.exec_time_ns
        kernel.last_results = res
    out = np.concatenate([res.results[c]["out"] for c in range(NCORES)], axis=0)
    return out
